# revision 1
# baseline (speedup 1.0000x reference)
"""Trainium2 Bass kernel for a 12-head attention block with post-softmax
additive per-head bias.

    qkv = x @ W_qkv                          x: [64, 196, 768]
    attn = softmax(q k^T / 8) + static_a     (bias added AFTER softmax)
    out = (attn @ v) @ W_proj + b_proj

Sharding: data-parallel over batch across 8 NeuronCores (8 batches each).
No collectives. Weights replicated.

Numerics: static_a (~0.9/entry) dominates the softmax probabilities
(~0.005..0.05/entry) in attn@v — the softmax path carries only ~1.2% of
the output norm, so q/k/S can run in fp8-e4m3 while v / A@v / proj stay
bf16. fp8 DoubleRow matmuls stream 2 K-tiles per instruction at 0.5
cyc/row (4x bf16 on the contraction-heavy qkT).

Per-core dataflow:
  prologue: q^T,k^T = W_qk^T @ x^T for ALL 8 local batches at once
            (fp8 DoubleRow, K=768 as 3 pairs of 128) -> fp8 SBUF
  per batch b:
    v(b)   = x_b @ W_v                 (bf16, 65-stride + ones column)
    S^T(b) = k q^T (fp8) -> exp (ACT, scale=1/(8*256), bias=-ln16) -> P~^T
    AV(b)  = A_h @ v  (bf16, per-head static bias term)
    U(b)   = P~ @ [v|1]  (ones column gives softmax row sums)
    O(b)   = U * (1/r) + AV            (DVE)
    O^T(b) via PE transpose -> attn_outT
  out = attn_out @ W_proj  (forward; b_proj added on host — it is zeros)
"""

import math
import os
import sys

_TRN_REPO = "/opt/trn_rl_repo"
if _TRN_REPO not in sys.path:
    sys.path.insert(0, _TRN_REPO)

import numpy as np
import ml_dtypes

import concourse.bass as bass
import concourse.tile as tile
from concourse import bacc, mybir
from concourse.bass import MemorySpace
from concourse.bass_utils import run_bass_kernel_spmd
from concourse.masks import make_identity

BF16 = mybir.dt.bfloat16
F32 = mybir.dt.float32
F8 = mybir.dt.float8e4
DR = mybir.MatmulPerfMode.DoubleRow

N_CORES = 8
BATCH = 64
B = BATCH // N_CORES  # 8 local batches per core
H = 12
D = 64
N = 196
C = 768
T = B * N  # 1568 local tokens
KC = 6  # contraction chunks of 128 over C=768
W8SCALE = 16.0  # host pre-scale on W_q,W_k before fp8 quantization
SCALE = (D ** -0.5) / (W8SCALE * W8SCALE)  # folded into exp
PSHIFT = 16.0  # P~ stored as exp(.)/PSHIFT to stay in fp8 range
EXPBIAS = -math.log(PSHIFT)

TPAD = 208  # aot per-batch column stride (196 + xbar padding)
# per-batch row chunks over N=196 (attention output rows)
NCH = [(0, 128), (128, 68)]
# per-batch key chunks: 2x98 so DoubleRow can pair them as K-tiles
MH = 98
# qkT free-dim chunks over T (PSUM bank = 512 f32)
QKC = [(i * 512, min(512, T - i * 512)) for i in range((T + 511) // 512)]

AluOp = mybir.AluOpType
ActFn = mybir.ActivationFunctionType


def _emit(nc: bass.Bass):
    # xt8: fp8 x^T in DR layout: xt8[p, kp, kt, t] = x[t, (2kp+kt)*128+p]
    # xtb: bf16 x^T: xtb[p, kc, t] = x[t, kc*128+p]
    # w8:  fp8 16*W_qk: w8[p, kp, kt, f] = 16*W_qkv[(2kp+kt)*128+p, f]
    # wv:  bf16 W_v: wv[p, kc, f] = W_qkv[kc*128+p, 2C+f]
    # at:  bf16 A^T: at[mc, p, h*N+n] = A[h, n, mc*128+p]
    # wp:  bf16 W_proj: wp[p, kc, o] = W_proj[kc*128+p, o]
    xt8_d = nc.declare_dram_parameter("xt8", [128, 6 * T], F8, isOutput=False)
    xtb_d = nc.declare_dram_parameter("xtb", [128, 6 * T], BF16, isOutput=False)
    w8_d = nc.declare_dram_parameter("w8", [128, 6 * 2 * C], F8, isOutput=False)
    wv_d = nc.declare_dram_parameter("wv", [128, KC * C], BF16, isOutput=False)
    at_d = nc.declare_dram_parameter("at", [2, 128, H * N], BF16, isOutput=False)
    wp_d = nc.declare_dram_parameter("wp", [128, KC * C], BF16, isOutput=False)
    # out^T: out_d[o, t] = out[t, o]; host transposes back
    out_d = nc.declare_dram_parameter("out", [C, T], BF16, isOutput=True)

    with tile.TileContext(nc) as tc:
        from contextlib import ExitStack

        with ExitStack() as stk:
            const = stk.enter_context(tc.tile_pool(name="const", bufs=1))
            wq = stk.enter_context(tc.tile_pool(name="wq", bufs=1))
            qkp = stk.enter_context(tc.tile_pool(name="qkp", bufs=1))
            vbp = stk.enter_context(tc.tile_pool(name="vbp", bufs=4))
            obp = stk.enter_context(tc.tile_pool(name="obp", bufs=3))
            ptp = stk.enter_context(tc.tile_pool(name="ptp", bufs=2))
            small = stk.enter_context(tc.tile_pool(name="small", bufs=3))
            outst = stk.enter_context(tc.tile_pool(name="outst", bufs=1))

            # ---- constants ----
            ebias = const.tile([128, 1], F32)
            nc.vector.memset(ebias, EXPBIAS)

            wv_sb = wq.tile([128, KC, C], BF16)
            xtb_sb = wq.tile([128, KC, T], BF16)
            at_sb = const.tile([128, 2, H * N], BF16)
            wp_sb = const.tile([128, KC, C], BF16)
            w8_sb = None
            xt8_sb = None
            aot_sb = None

            # q^T/k^T fp8, feature-major, all batches: q8[p, ft, t]
            q8 = qkp.tile([128, KC, T], F8)
            k8 = qkp.tile([128, KC, T], F8)
            # DoubleRow refold: q8f[q, ft, (h%2)*2+dh, t] = q^T row
            # ft*128 + ((h%2)*2+dh)*32 + q -- the two 32-row d-halves of a
            # head become the K-tile pair, always at base partitions 0..31
            q8f = qkp.tile([32, KC, 4, T], F8)
            k8f = qkp.tile([32, KC, 4, T], F8)

            big = {}
            vb_t = {}
            v8_t = {}
            ob_t = {}
            pt_t = {}
            av_t = {}
            otb_t = {}

            def emit_qkT(psu):
                # q^T,k^T = W_qk^T @ x^T, fp8 DoubleRow, K=768 as 3 pairs.
                # qc0 in kp-outer waves (PE chews partial accumulations while
                # xt8 streams in), then ft-outer so each tile's refold DMAs
                # issue as early as possible. st consumes head-pair tiles in
                # order, so finish (q, k) tile pairs together.
                ftorder = list(range(12))
                toff0, tlen0 = QKC[0]
                for fts in (ftorder[0:4], ftorder[4:8], ftorder[8:12]):
                    pqs = {}
                    for ft in fts:
                        pq_w = psu.tile([128, 512], F32, tag="ps")
                        pqs[ft] = pq_w
                    for kp in range(3):
                        for ft in fts:
                            nc.tensor.matmul(
                                pqs[ft][:, 0:tlen0],
                                lhsT=big['w8'][
                                    :, ft // 2, kp, :,
                                    (ft % 2) * 128 : (ft % 2) * 128 + 128,
                                ],
                                rhs=big['xt8'][:, kp, :, toff0 : toff0 + tlen0],
                                start=(kp == 0),
                                stop=(kp == 2),
                                perf_mode=DR,
                            )
                    for ft in fts:
                        dst = q8 if ft < 6 else k8
                        c = ft % 6
                        if ft % 2 == 0:
                            nc.vector.tensor_copy(
                                dst[:, c, toff0 : toff0 + tlen0],
                                pqs[ft][:, 0:tlen0],
                            )
                        else:
                            nc.scalar.copy(
                                dst[:, c, toff0 : toff0 + tlen0],
                                pqs[ft][:, 0:tlen0],
                            )
                for ft in ftorder:
                    dst = q8 if ft < 6 else k8
                    c = ft % 6
                    for qc, (toff, tlen) in enumerate(QKC):
                        if qc == 0:
                            continue
                        pq_w = psu.tile([128, 512], F32, tag="ps")
                        for kp in range(3):
                            nc.tensor.matmul(
                                pq_w[:, 0:tlen],
                                lhsT=big['w8'][
                                    :, ft // 2, kp, :,
                                    (ft % 2) * 128 : (ft % 2) * 128 + 128,
                                ],
                                rhs=big['xt8'][:, kp, :, toff : toff + tlen],
                                start=(kp == 0),
                                stop=(kp == 2),
                                perf_mode=DR,
                            )
                        if ft % 2 == 0:
                            nc.vector.tensor_copy(
                                dst[:, c, toff : toff + tlen], pq_w[:, 0:tlen]
                            )
                        else:
                            nc.scalar.copy(
                                dst[:, c, toff : toff + tlen], pq_w[:, 0:tlen]
                            )
                    # refold [32, 4, T] once each of q/k completes: ONE
                    # big DMA per partition-quadrant (SP-queue dispatch is
                    # ~600ns per DMA, so 8 beats 48), split across the SP
                    # and ACT queues
                    if ft == 5 or ft == 11:
                        fold = q8f if ft == 5 else k8f
                        srcq = q8 if ft == 5 else k8
                        for j in range(4):
                            eng = nc.sync if j % 2 == 0 else nc.scalar
                            eng.dma_start(
                                out=fold[:, :, j, :],
                                in_=srcq[32 * j : 32 * j + 32, :, :],
                            )

            def v_group(b, g, psu):
                if g == 0:
                    vb_new = vbp.tile([128, 2, C], BF16, tag="vb")
                    v8_new = vbp.tile([128, 2, H * 65], F8, tag="v8")
                    vb_t[b] = vb_new
                    v8_t[b] = v8_new
                    # ones column at 65-stride: U's 65th output column is
                    # the softmax row sum
                    nc.gpsimd.memset(
                        v8_new.rearrange("p a (h x) -> p a h x", h=H)[
                            :, :, :, 64:65
                        ],
                        1.0,
                    )
                vb, v8 = vb_t[b], v8_t[b]
                mc, ns = g // 2, g % 2
                moff = mc * MH
                ps = psu.tile([128, 512], F32, tag="ps")
                for kc in range(KC):
                    nc.tensor.matmul(
                        ps[0:MH, 0:384],
                        lhsT=xtb_sb[:, kc, b * N + moff : b * N + moff + MH],
                        rhs=wv_sb[:, kc, ns * 384 : (ns + 1) * 384],
                        start=(kc == 0),
                        stop=(kc == KC - 1),
                    )
                nc.vector.tensor_copy(
                    vb[0:MH, mc, ns * 384 : (ns + 1) * 384], ps[0:MH, 0:384]
                )
                # fp8 twin from SBUF on the idle gpsimd engine
                # (walrus can't codegen a gpsimd PSUM read, and this
                # frees the PSUM slot as soon as the DVE copy lands)
                nc.gpsimd.tensor_copy(
                    v8[0:MH, mc, :].rearrange("p (h x) -> p h x", h=H)[
                        :, ns * 6 : (ns + 1) * 6, 0:64
                    ],
                    vb[0:MH, mc, ns * 384 : (ns + 1) * 384].rearrange(
                        "p (h x) -> p h x", h=6
                    ),
                )

            def emit_v(b, psu):
                for g in range(4):
                    v_group(b, g, psu)

            def open_batch(b):
                ob = obp.tile([128, 2, C], BF16, tag="ob")
                ob_t[b] = ob
                # zero the xbar pad rows (68..79); base partition must be a
                # multiple of 32, rows 64..67 are rewritten by the uo add
                nc.gpsimd.memset(ob[64:80, 1, :], 0.0)
                pt = ptp.tile([128, 2, H * N], F8, tag="pt")
                pt_t[b] = pt
                av_t[b] = {}

            def st_unit(b, mc, hg, psSt):
                pt = pt_t[b]
                moff = mc * MH
                ps = psSt.tile([128, 1024], F32, tag="pst")
                for hh in range(4):
                    h = hg * 4 + hh
                    off = (hh // 2) * 512 + (hh % 2) * 196
                    blk = (h % 2) * 2
                    nc.tensor.matmul(
                        ps[0:MH, off : off + 196],
                        lhsT=k8f[
                            :, h // 2, blk : blk + 2,
                            b * N + moff : b * N + moff + MH,
                        ],
                        rhs=q8f[:, h // 2, blk : blk + 2, b * N : b * N + N],
                        start=True,
                        stop=True,
                        perf_mode=DR,
                    )
                src_v = ps.rearrange("p (k x) -> p k x", k=2)[
                    0:MH, :, 0:392
                ].rearrange("p k (h n) -> p k h n", h=2)
                dst_v = pt[0:MH, mc, hg * 4 * N : (hg + 1) * 4 * N].rearrange(
                    "p (k h n) -> p k h n", k=2, h=2
                )
                nc.scalar.activation(
                    dst_v, src_v, ActFn.Exp, bias=ebias[0:MH, :], scale=SCALE
                )

            def av_unit(b, nc_i, half, psu):
                vb = vb_t[b]
                ob = ob_t[b]
                noff, nlen = NCH[nc_i]
                av = psu.tile([128, 512], F32, tag="ps")
                for ho in range(6):
                    h = half * 6 + ho
                    for mc in range(2):
                        nc.tensor.matmul(
                            av[0:nlen, ho * 64 : ho * 64 + 64],
                            lhsT=at_sb[
                                0:MH, mc, h * N + noff : h * N + noff + nlen
                            ],
                            rhs=vb[0:MH, mc, h * 64 : h * 64 + 64],
                            start=(mc == 0),
                            stop=(mc == 1),
                        )
                nc.scalar.copy(
                    ob[0:nlen, nc_i, half * 384 : (half + 1) * 384],
                    av[0:nlen, 0:384],
                )

            def emit_uo(b, psu, ncs=(0, 1)):
                # U = P~ @ v via fp8 DoubleRow (the two 98-key chunks are the
                # K-tile pair -> one matmul per head), row sums r as rank-1
                # DR matmuls into the same PSUM tile, then
                # ob = U*(1/r) + AV(psum).
                v8 = v8_t[b]
                pt = pt_t[b]
                ob = ob_t[b]
                for nc_i in ncs:
                    noff, nlen = NCH[nc_i]
                    rec = small.tile([128, H], F32, tag="rec")
                    tmp = small.tile([128, C], F32, tag="tmp")
                    for half in range(2):
                        uph = psu.tile([128, 512], F32, tag="ps")
                        for ho in range(6):
                            h = half * 6 + ho
                            nc.tensor.matmul(
                                uph[0:nlen, ho * 65 : ho * 65 + 65],
                                lhsT=pt[0:MH, :, h * N + noff : h * N + noff + nlen],
                                rhs=v8[0:MH, :, h * 65 : h * 65 + 65],
                                start=True,
                                stop=True,
                                perf_mode=DR,
                            )
                        upv = uph[0:nlen, 0:390].rearrange("p (h x) -> p h x", h=6)
                        recv = rec[0:nlen, half * 6 : half * 6 + 6, None]
                        nc.vector.reciprocal(recv, upv[:, :, 64:65])
                        nc.vector.tensor_tensor(
                            tmp[0:nlen, half * 384 : (half + 1) * 384].rearrange(
                                "p (h c) -> p h c", h=6
                            ),
                            upv[:, :, 0:64],
                            recv.to_broadcast((nlen, 6, 64)),
                            AluOp.mult,
                        )
                        nc.vector.tensor_tensor(
                            ob[0:nlen, nc_i, half * 384 : (half + 1) * 384],
                            tmp[0:nlen, half * 384 : (half + 1) * 384],
                            ob[0:nlen, nc_i, half * 384 : (half + 1) * 384],
                            AluOp.add,
                        )

            def emit_tr(b, chunk):
                # O [n, c] -> O^T [c, n] on the DMA xbar (16x128 tiles).
                # chunk 1 is 68 rows padded to 80; pad rows are zeroed at
                # batch start, pad columns land in aot's per-batch slack.
                ob = ob_t[b]
                if chunk == 0:
                    nc.sync.dma_start_transpose(
                        big['aot'][:, :, b * TPAD : b * TPAD + 128],
                        ob[0:128, 0, :],
                    )
                else:
                    nc.sync.dma_start_transpose(
                        big['aot'][:, :, b * TPAD + 128 : b * TPAD + 208],
                        ob[0:80, 1, :],
                    )

            def proj_unit(b, o, psu, coff=0, clen=N):
                # out^T[o, t] for batch b; stage bf16 rows, ship every 2nd
                # batch so the out DMAs stay above the descriptor floor
                if b % 2 == 0 and coff == 0 and o == 0:
                    otb_new = outst.tile([128, KC, 2 * N], BF16, tag="otb")
                    otb_t[b // 2] = otb_new
                otb = otb_t[b // 2]
                if True:
                    pp = psu.tile([128, 512], F32, tag="ps")
                    for kc in range(KC):
                        nc.tensor.matmul(
                            pp[:, 0:clen],
                            lhsT=wp_sb[:, kc, o * 128 : (o + 1) * 128],
                            rhs=big['aot'][
                                :, kc, b * TPAD + coff : b * TPAD + coff + clen
                            ],
                            start=(kc == 0),
                            stop=(kc == KC - 1),
                        )
                    dsl = otb[:, o, (b % 2) * N + coff : (b % 2) * N + coff + clen]
                    nc.vector.tensor_copy(dsl, pp[:, 0:clen])
                    if b % 2 == 1 and coff + clen == N:
                        nc.sync.dma_start(
                            out=out_d[
                                o * 128 : (o + 1) * 128, (b - 1) * N : (b + 1) * N
                            ],
                            in_=otb[:, o, :],
                        )

            def emit_proj(b, psu, coff=0, clen=N):
                for o in range(KC):
                    proj_unit(b, o, psu, coff, clen)

            with (
                tc.tile_pool(name="psu", bufs=4, space=MemorySpace.PSUM) as psu,
                tc.tile_pool(name="psSt", bufs=2, space=MemorySpace.PSUM) as psSt,
            ):
                with tc.tile_pool(name="xw", bufs=1) as xw:
                    w8_sb = xw.tile([128, 6, 3, 2, 256], F8)
                    xt8_sb = xw.tile([128, 3, 2, T], F8)
                    big['w8'] = w8_sb
                    big['xt8'] = xt8_sb
                    # qc0's wave8 needs xt8 (all kp) + w8 ftp0..3; the rest
                    # in first-use order so v(0) can start while qkT drains
                    nc.sync.dma_start(
                        out=w8_sb[:, 0].rearrange("p a b c -> p (a b c)"),
                        in_=w8_d[:, 0:1536],
                    )
                    for kp in range(3):
                        nc.sync.dma_start(
                            out=xt8_sb[:, kp, :, :].rearrange("p a b -> p (a b)"),
                            in_=xt8_d[:, kp * 2 * T : (kp + 1) * 2 * T],
                        )
                    for ftp in range(1, 6):
                        nc.sync.dma_start(
                            out=w8_sb[:, ftp].rearrange("p a b c -> p (a b c)"),
                            in_=w8_d[:, ftp * 1536 : (ftp + 1) * 1536],
                        )
                    nc.sync.dma_start(
                        out=xtb_sb.rearrange("p a b -> p (a b)"),
                        in_=xtb_d[:, :],
                    )
                    nc.sync.dma_start(
                        out=wv_sb.rearrange("p a b -> p (a b)"), in_=wv_d[:, :]
                    )
                    for mc in range(2):
                        nc.sync.dma_start(
                            out=at_sb[:, mc, :], in_=at_d[mc, :, :]
                        )

                    emit_qkT(psu)
                    emit_v(0, psu)
                    emit_v(1, psu)

                aotp = stk.enter_context(tc.tile_pool(name="aotp", bufs=1))
                aot_new = aotp.tile([128, KC, B * TPAD], BF16)
                big['aot'] = aot_new
                for b in range(B):
                    open_batch(b)
                    # st groups with av blocks interleaved (exp paces PSUM
                    # recycling); uo runs one batch behind so it never waits
                    # on this batch's exp queue
                    order = [
                        ("av", 0, 0), ("st", 0, 0), ("av", 0, 1), ("st", 0, 1),
                        ("av", 1, 0), ("st", 0, 2), ("av", 1, 1), ("st", 1, 0),
                        ("vg", 0, 0), ("st", 1, 1), ("vg", 1, 0), ("st", 1, 2),
                        ("vg", 2, 0), ("vg", 3, 0),
                    ]
                    for kind, a1, a2 in order:
                        if kind == "st":
                            st_unit(b, a1, a2, psSt)
                        elif kind == "av":
                            av_unit(b, a1, a2, psu)
                        elif b + 2 < B:
                            v_group(b + 2, a1, psu)
                    if b >= 1:
                        emit_uo(b - 1, psu, ncs=(0,))
                        emit_tr(b - 1, 0)
                        emit_uo(b - 1, psu, ncs=(1,))
                        emit_tr(b - 1, 1)
                    if b >= 2:
                        emit_proj(b - 2, psu)
                    if b == 0:
                        nc.sync.dma_start(
                            out=wp_sb.rearrange("p a b -> p (a b)"),
                            in_=wp_d[:, :],
                        )
                emit_uo(B - 1, psu, ncs=(0,))
                emit_tr(B - 1, 0)
                emit_uo(B - 1, psu, ncs=(1,))
                emit_tr(B - 1, 1)
                emit_proj(B - 2, psu)
                for o in range(KC):
                    proj_unit(B - 1, o, psu, coff=0, clen=128)
                    proj_unit(B - 1, o, psu, coff=128, clen=68)

    return nc


_CACHE: dict = {}


def _get_module():
    if "nc" not in _CACHE:
        nc = bacc.Bacc(None, target_bir_lowering=False)
        _emit(nc)
        nc.compile()
        _CACHE["nc"] = nc
    return _CACHE["nc"]


def prepare_core_inputs(x_shard, W_qkv, static_a, W_proj):
    """Build the per-core input map from a [B, N, C] batch shard."""
    bf = ml_dtypes.bfloat16
    f8 = ml_dtypes.float8_e4m3
    xT = np.ascontiguousarray(
        x_shard.reshape(T, C).T
    )  # [768, 1568]
    xt8 = (
        xT.reshape(3, 2, 128, T).transpose(2, 0, 1, 3).reshape(128, 6 * T)
    ).astype(f8)
    xtb = (
        xT.reshape(KC, 128, T).transpose(1, 0, 2).reshape(128, KC * T)
    ).astype(bf)
    return dict(xt8=np.ascontiguousarray(xt8), xtb=np.ascontiguousarray(xtb))


def prepare_shared_inputs(W_qkv, static_a, W_proj):
    bf = ml_dtypes.bfloat16
    f8 = ml_dtypes.float8_e4m3
    # w8[p, ftp, kp, kt, fo] = 16*W_qkv[(2kp+kt)*128+p, ftp*256+fo]
    w8 = (
        (W_qkv[:, : 2 * C] * W8SCALE)
        .reshape(3, 2, 128, 6, 256)
        .transpose(2, 3, 0, 1, 4)
        .reshape(128, 6 * 2 * C)
    ).astype(f8)
    wv = (
        W_qkv[:, 2 * C :].reshape(KC, 128, C).transpose(1, 0, 2).reshape(128, KC * C)
    ).astype(bf)
    A = static_a[0]  # [H, N, N]
    Am = np.ascontiguousarray(A.transpose(2, 0, 1))  # [m, H, n]
    at_arr = np.zeros((2, 128, H, N), dtype=np.float32)
    at_arr[0, 0:98] = Am[0:98]
    at_arr[1, 0:98] = Am[98:196]
    at = at_arr.reshape(2, 128, H * N).astype(bf)
    wp = (
        W_proj.reshape(KC, 128, C).transpose(1, 0, 2).reshape(128, KC * C)
    ).astype(bf)
    return dict(
        w8=np.ascontiguousarray(w8),
        wv=np.ascontiguousarray(wv),
        at=np.ascontiguousarray(at),
        wp=np.ascontiguousarray(wp),
    )


_last_results = None


def kernel(x, W_qkv, static_a, W_proj, b_proj):
    global _last_results
    x = np.asarray(x, dtype=np.float32)
    W_qkv = np.asarray(W_qkv, dtype=np.float32)
    static_a = np.asarray(static_a, dtype=np.float32)
    W_proj = np.asarray(W_proj, dtype=np.float32)
    b_proj = np.asarray(b_proj, dtype=np.float32)

    shared = prepare_shared_inputs(W_qkv, static_a, W_proj)
    in_maps = []
    for i in range(N_CORES):
        m = dict(shared)
        m.update(prepare_core_inputs(x[i * B : (i + 1) * B], W_qkv, static_a, W_proj))
        in_maps.append(m)

    nc = _get_module()
    res = run_bass_kernel_spmd(nc, in_maps, core_ids=list(range(N_CORES)))
    _last_results = res
    out = np.concatenate(
        [
            np.asarray(r["out"]).astype(np.float32).reshape(C, B, N).transpose(1, 2, 0)
            for r in res.results
        ],
        axis=0,
    )
    out = np.ascontiguousarray(out)
    if b_proj.any():
        out = out + b_proj.reshape(1, 1, C)
    return out



# revision 8
# speedup vs baseline: 1.0860x; 1.0860x over previous
"""Trainium2 Bass kernel for a 12-head attention block with post-softmax
additive per-head bias.

    qkv = x @ W_qkv                          x: [64, 196, 768]
    attn = softmax(q k^T / 8) + static_a     (bias added AFTER softmax)
    out = (attn @ v) @ W_proj + b_proj

Sharding: data-parallel over batch across 8 NeuronCores (8 batches each).
No collectives. Weights replicated.

Numerics: static_a (~0.9/entry) dominates the softmax probabilities
(~0.005..0.05/entry) in attn@v — the softmax path carries only ~1.2% of
the output norm, so q/k/S can run in fp8-e4m3 while v / A@v / proj stay
bf16. fp8 DoubleRow matmuls stream 2 K-tiles per instruction at 0.5
cyc/row (4x bf16 on the contraction-heavy qkT).

Per-core dataflow:
  prologue: q^T,k^T = W_qk^T @ x^T for ALL 8 local batches at once
            (fp8 DoubleRow, K=768 as 3 pairs of 128) -> fp8 SBUF
  per batch b:
    v(b)   = x_b @ W_v                 (bf16, 65-stride + ones column)
    S^T(b) = k q^T (fp8) -> exp (ACT, scale=1/(8*256), bias=-ln16) -> P~^T
    AV(b)  = A_h @ v  (bf16, per-head static bias term)
    U(b)   = P~ @ [v|1]  (ones column gives softmax row sums)
    O(b)   = U * (1/r) + AV            (DVE)
    O^T(b) via PE transpose -> attn_outT
  out = attn_out @ W_proj  (forward; b_proj added on host — it is zeros)
"""

import math
import os
import sys

_TRN_REPO = "/opt/trn_rl_repo"
if _TRN_REPO not in sys.path:
    sys.path.insert(0, _TRN_REPO)

import numpy as np
import ml_dtypes

import concourse.bass as bass
import concourse.tile as tile
from concourse import bacc, mybir
from concourse.bass import MemorySpace
from concourse.bass_utils import run_bass_kernel_spmd
from concourse.masks import make_identity

BF16 = mybir.dt.bfloat16
F32 = mybir.dt.float32
F8 = mybir.dt.float8e4
DR = mybir.MatmulPerfMode.DoubleRow

N_CORES = 8
BATCH = 64
B = BATCH // N_CORES  # 8 local batches per core
H = 12
D = 64
N = 196
C = 768
T = B * N  # 1568 local tokens
KC = 6  # contraction chunks of 128 over C=768
W8SCALE = 16.0  # host pre-scale on W_q,W_k before fp8 quantization
SCALE = (D ** -0.5) / (W8SCALE * W8SCALE)  # folded into exp
PSHIFT = 16.0  # P~ stored as exp(.)/PSHIFT to stay in fp8 range
EXPBIAS = -math.log(PSHIFT)

TPAD = 208  # aot per-batch column stride (196 + xbar padding)
# per-batch row chunks over N=196 (attention output rows)
NCH = [(0, 128), (128, 68)]
# per-batch key chunks: 2x98 so DoubleRow can pair them as K-tiles
MH = 98
# qkT free-dim chunks over T (PSUM bank = 512 f32)
QKC = [(i * 512, min(512, T - i * 512)) for i in range((T + 511) // 512)]

AluOp = mybir.AluOpType
ActFn = mybir.ActivationFunctionType


def _emit(nc: bass.Bass):
    # xt8: fp8 x^T in DR layout: xt8[p, kp, kt, t] = x[t, (2kp+kt)*128+p]
    # xtb: bf16 x^T: xtb[p, kc, t] = x[t, kc*128+p]
    # w8:  fp8 16*W_qk: w8[p, kp, kt, f] = 16*W_qkv[(2kp+kt)*128+p, f]
    # wv:  bf16 W_v: wv[p, kc, f] = W_qkv[kc*128+p, 2C+f]
    # at:  bf16 A^T: at[mc, p, h*N+n] = A[h, n, mc*128+p]
    # wp:  bf16 W_proj: wp[p, kc, o] = W_proj[kc*128+p, o]
    xt8_d = nc.declare_dram_parameter("xt8", [128, 6 * T], F8, isOutput=False)
    xtb_d = nc.declare_dram_parameter("xtb", [128, 6 * T], BF16, isOutput=False)
    w8_d = nc.declare_dram_parameter("w8", [128, 6 * 2 * C], F8, isOutput=False)
    wv_d = nc.declare_dram_parameter("wv", [128, KC * C], BF16, isOutput=False)
    at_d = nc.declare_dram_parameter("at", [2, 128, H * N], BF16, isOutput=False)
    wp_d = nc.declare_dram_parameter("wp", [128, KC * C], BF16, isOutput=False)
    # out^T: out_d[o, t] = out[t, o]; host transposes back
    out_d = nc.declare_dram_parameter("out", [C, T], BF16, isOutput=True)

    with tile.TileContext(nc) as tc:
        from contextlib import ExitStack

        with ExitStack() as stk:
            const = stk.enter_context(tc.tile_pool(name="const", bufs=1))
            wq = stk.enter_context(tc.tile_pool(name="wq", bufs=1))
            qkp = stk.enter_context(tc.tile_pool(name="qkp", bufs=1))
            vbp = stk.enter_context(tc.tile_pool(name="vbp", bufs=4))
            obp = stk.enter_context(tc.tile_pool(name="obp", bufs=3))
            ptp = stk.enter_context(tc.tile_pool(name="ptp", bufs=2))
            small = stk.enter_context(tc.tile_pool(name="small", bufs=3))
            outst = stk.enter_context(tc.tile_pool(name="outst", bufs=2))

            # ---- constants ----
            ebias = const.tile([128, 1], F32)
            nc.vector.memset(ebias, EXPBIAS)

            wv_sb = wq.tile([128, KC, C], BF16)
            xtb_sb = wq.tile([128, KC, T], BF16)
            at_sb = const.tile([128, 2, H * N], BF16)
            wp_sb = const.tile([128, KC, C], BF16)
            w8_sb = None
            xt8_sb = None
            aot_sb = None

            # q^T/k^T fp8, all batches, in DR-pair layout via host-side
            # W-column permutation: head h, dim d lives at
            # c = 2*(h//4) + d//32, p = 32*(h%4) + d%32, so st's lhsT/rhs
            # [32, 2, m] slices read q8/k8 directly (no refold DMAs).
            q8 = qkp.tile([128, KC, T], F8)
            k8 = qkp.tile([128, KC, T], F8)

            big = {}
            vb_t = {}
            v8_t = {}
            ob_t = {}
            pt_t = {}
            av_t = {}
            otb_t = {}

            def emit_qkT(psu):
                # q^T,k^T = W_qk^T @ x^T, fp8 DoubleRow, K=768 as 3 pairs.
                # qc0 in kp-outer waves (PE chews partial accumulations while
                # xt8 streams in), then ft-outer so each tile's refold DMAs
                # issue as early as possible. st consumes head-pair tiles in
                # order, so finish (q, k) tile pairs together.
                ftorder = list(range(12))
                toff0, tlen0 = QKC[0]
                for fts in (ftorder[0:4], ftorder[4:8], ftorder[8:12]):
                    pqs = {}
                    for ft in fts:
                        pq_w = psu.tile([128, 512], F32, tag="ps")
                        pqs[ft] = pq_w
                    for kp in range(3):
                        for ft in fts:
                            nc.tensor.matmul(
                                pqs[ft][:, 0:tlen0],
                                lhsT=big['w8'][
                                    :, ft // 2, kp, :,
                                    (ft % 2) * 128 : (ft % 2) * 128 + 128,
                                ],
                                rhs=big['xt8'][:, kp, :, toff0 : toff0 + tlen0],
                                start=(kp == 0),
                                stop=(kp == 2),
                                perf_mode=DR,
                            )
                    for ft in fts:
                        dst = q8 if ft < 6 else k8
                        c = ft % 6
                        if ft % 2 == 0:
                            nc.vector.tensor_copy(
                                dst[:, c, toff0 : toff0 + tlen0],
                                pqs[ft][:, 0:tlen0],
                            )
                        else:
                            nc.scalar.copy(
                                dst[:, c, toff0 : toff0 + tlen0],
                                pqs[ft][:, 0:tlen0],
                            )
                for ft in ftorder:
                    dst = q8 if ft < 6 else k8
                    c = ft % 6
                    for qc, (toff, tlen) in enumerate(QKC):
                        if qc == 0:
                            continue
                        pq_w = psu.tile([128, 512], F32, tag="ps")
                        for kp in range(3):
                            nc.tensor.matmul(
                                pq_w[:, 0:tlen],
                                lhsT=big['w8'][
                                    :, ft // 2, kp, :,
                                    (ft % 2) * 128 : (ft % 2) * 128 + 128,
                                ],
                                rhs=big['xt8'][:, kp, :, toff : toff + tlen],
                                start=(kp == 0),
                                stop=(kp == 2),
                                perf_mode=DR,
                            )
                        if ft % 2 == 0:
                            nc.vector.tensor_copy(
                                dst[:, c, toff : toff + tlen], pq_w[:, 0:tlen]
                            )
                        else:
                            nc.scalar.copy(
                                dst[:, c, toff : toff + tlen], pq_w[:, 0:tlen]
                            )

            def v_group(b, g, psu):
                if g == 0:
                    vb_new = vbp.tile([128, 2, C], BF16, tag="vb")
                    v8_new = vbp.tile([128, 2, H * 65], F8, tag="v8")
                    vb_t[b] = vb_new
                    v8_t[b] = v8_new
                    # ones column at 65-stride: U's 65th output column is
                    # the softmax row sum
                    nc.gpsimd.memset(
                        v8_new.rearrange("p a (h x) -> p a h x", h=H)[
                            :, :, :, 64:65
                        ],
                        1.0,
                    )
                vb, v8 = vb_t[b], v8_t[b]
                mc, ns = g // 2, g % 2
                moff = mc * MH
                ps = psu.tile([128, 512], F32, tag="ps")
                for kc in range(KC):
                    nc.tensor.matmul(
                        ps[0:MH, 0:384],
                        lhsT=xtb_sb[:, kc, b * N + moff : b * N + moff + MH],
                        rhs=wv_sb[:, kc, ns * 384 : (ns + 1) * 384],
                        start=(kc == 0),
                        stop=(kc == KC - 1),
                    )
                nc.vector.tensor_copy(
                    vb[0:MH, mc, ns * 384 : (ns + 1) * 384], ps[0:MH, 0:384]
                )
                # fp8 twin from SBUF on the idle gpsimd engine
                # (walrus can't codegen a gpsimd PSUM read, and this
                # frees the PSUM slot as soon as the DVE copy lands)
                nc.gpsimd.tensor_copy(
                    v8[0:MH, mc, :].rearrange("p (h x) -> p h x", h=H)[
                        :, ns * 6 : (ns + 1) * 6, 0:64
                    ],
                    vb[0:MH, mc, ns * 384 : (ns + 1) * 384].rearrange(
                        "p (h x) -> p h x", h=6
                    ),
                )

            def emit_v(b, psu):
                for g in range(4):
                    v_group(b, g, psu)

            def open_batch(b):
                ob = obp.tile([128, 2, C], BF16, tag="ob")
                ob_t[b] = ob
                # zero the xbar pad rows (68..79); base partition must be a
                # multiple of 32, rows 64..67 are rewritten by the uo add
                nc.gpsimd.memset(ob[64:80, 1, :], 0.0)
                pt = ptp.tile([128, 2, H * N], F8, tag="pt")
                pt_t[b] = pt
                av_t[b] = {}

            def st_unit(b, mc, hg, psSt):
                pt = pt_t[b]
                moff = mc * MH
                ps = psSt.tile([128, 1024], F32, tag="pst")
                for hh in range(4):
                    h = hg * 4 + hh
                    off = (hh // 2) * 512 + (hh % 2) * 196
                    qb = 32 * (h % 4)
                    c0 = 2 * (h // 4)
                    nc.tensor.matmul(
                        ps[0:MH, off : off + 196],
                        lhsT=k8[
                            qb : qb + 32, c0 : c0 + 2,
                            b * N + moff : b * N + moff + MH,
                        ],
                        rhs=q8[qb : qb + 32, c0 : c0 + 2, b * N : b * N + N],
                        start=True,
                        stop=True,
                        perf_mode=DR,
                        tile_position=(qb, 0),
                    )
                src_v = ps.rearrange("p (k x) -> p k x", k=2)[
                    0:MH, :, 0:392
                ].rearrange("p k (h n) -> p k h n", h=2)
                dst_v = pt[0:MH, mc, hg * 4 * N : (hg + 1) * 4 * N].rearrange(
                    "p (k h n) -> p k h n", k=2, h=2
                )
                nc.scalar.activation(
                    dst_v, src_v, ActFn.Exp, bias=ebias[0:MH, :], scale=SCALE
                )

            def av_unit(b, nc_i, half, psu):
                vb = vb_t[b]
                ob = ob_t[b]
                noff, nlen = NCH[nc_i]
                av = psu.tile([128, 512], F32, tag="ps")
                for ho in range(6):
                    h = half * 6 + ho
                    for mc in range(2):
                        nc.tensor.matmul(
                            av[0:nlen, ho * 64 : ho * 64 + 64],
                            lhsT=at_sb[
                                0:MH, mc, h * N + noff : h * N + noff + nlen
                            ],
                            rhs=vb[0:MH, mc, h * 64 : h * 64 + 64],
                            start=(mc == 0),
                            stop=(mc == 1),
                        )
                nc.scalar.copy(
                    ob[0:nlen, nc_i, half * 384 : (half + 1) * 384],
                    av[0:nlen, 0:384],
                )

            def emit_uo(b, psu, ncs=(0, 1)):
                # U = P~ @ v via fp8 DoubleRow (the two 98-key chunks are the
                # K-tile pair -> one matmul per head), row sums r as rank-1
                # DR matmuls into the same PSUM tile, then
                # ob = U*(1/r) + AV(psum).
                v8 = v8_t[b]
                pt = pt_t[b]
                ob = ob_t[b]
                for nc_i in ncs:
                    noff, nlen = NCH[nc_i]
                    rec = small.tile([128, H], F32, tag="rec")
                    tmp = small.tile([128, C], F32, tag="tmp")
                    for half in range(2):
                        uph = psu.tile([128, 512], F32, tag="ps")
                        for ho in range(6):
                            h = half * 6 + ho
                            nc.tensor.matmul(
                                uph[0:nlen, ho * 65 : ho * 65 + 65],
                                lhsT=pt[0:MH, :, h * N + noff : h * N + noff + nlen],
                                rhs=v8[0:MH, :, h * 65 : h * 65 + 65],
                                start=True,
                                stop=True,
                                perf_mode=DR,
                            )
                        upv = uph[0:nlen, 0:390].rearrange("p (h x) -> p h x", h=6)
                        recv = rec[0:nlen, half * 6 : half * 6 + 6, None]
                        nc.vector.reciprocal(recv, upv[:, :, 64:65])
                        nc.vector.tensor_tensor(
                            tmp[0:nlen, half * 384 : (half + 1) * 384].rearrange(
                                "p (h c) -> p h c", h=6
                            ),
                            upv[:, :, 0:64],
                            recv.to_broadcast((nlen, 6, 64)),
                            AluOp.mult,
                        )
                        nc.vector.tensor_tensor(
                            ob[0:nlen, nc_i, half * 384 : (half + 1) * 384],
                            tmp[0:nlen, half * 384 : (half + 1) * 384],
                            ob[0:nlen, nc_i, half * 384 : (half + 1) * 384],
                            AluOp.add,
                        )

            def emit_tr(b, chunk):
                # O [n, c] -> O^T [c, n] on the DMA xbar (16x128 tiles).
                # chunk 1 is 68 rows padded to 80; pad rows are zeroed at
                # batch start, pad columns land in aot's per-batch slack.
                ob = ob_t[b]
                if chunk == 0:
                    nc.sync.dma_start_transpose(
                        big['aot'][:, :, b * TPAD : b * TPAD + 128],
                        ob[0:128, 0, :],
                    )
                else:
                    nc.sync.dma_start_transpose(
                        big['aot'][:, :, b * TPAD + 128 : b * TPAD + 208],
                        ob[0:80, 1, :],
                    )

            def proj_unit(b, o, psu, coff=0, clen=N):
                # out^T[o, t] for batch b; stage bf16 rows, ship every 2nd
                # batch so the out DMAs stay above the descriptor floor
                if b % 2 == 0 and coff == 0 and o == 0:
                    otb_new = outst.tile([128, KC, 2 * N], BF16, tag="otb")
                    otb_t[b // 2] = otb_new
                otb = otb_t[b // 2]
                if True:
                    pp = psu.tile([128, 512], F32, tag="ps")
                    for kc in range(KC):
                        nc.tensor.matmul(
                            pp[:, 0:clen],
                            lhsT=wp_sb[:, kc, o * 128 : (o + 1) * 128],
                            rhs=big['aot'][
                                :, kc, b * TPAD + coff : b * TPAD + coff + clen
                            ],
                            start=(kc == 0),
                            stop=(kc == KC - 1),
                        )
                    dsl = otb[:, o, (b % 2) * N + coff : (b % 2) * N + coff + clen]
                    nc.vector.tensor_copy(dsl, pp[:, 0:clen])
                    if b % 2 == 1 and coff + clen == N:
                        nc.sync.dma_start(
                            out=out_d[
                                o * 128 : (o + 1) * 128, (b - 1) * N : (b + 1) * N
                            ],
                            in_=otb[:, o, :],
                        )

            def emit_proj(b, psu, coff=0, clen=N):
                for o in range(KC):
                    proj_unit(b, o, psu, coff, clen)

            with tc.tile_pool(name="xw", bufs=1) as xw:
                w8_sb = xw.tile([128, 6, 3, 2, 256], F8)
                xt8_sb = xw.tile([128, 3, 2, T], F8)
                big['w8'] = w8_sb
                big['xt8'] = xt8_sb
                # first wave needs w8 ftp0/1 + xt8; spread the loads over
                # the three DMA-capable queues (SP, ACT, Pool) in critical
                # order — transfer time is charged serially per queue
                nc.sync.dma_start(
                    out=w8_sb[:, 0].rearrange("p a b c -> p (a b c)"),
                    in_=w8_d[:, 0:1536],
                )
                nc.scalar.dma_start(
                    out=xt8_sb[:, 0, :, :].rearrange("p a b -> p (a b)"),
                    in_=xt8_d[:, 0 : 2 * T],
                )
                nc.gpsimd.dma_start(
                    out=xt8_sb[:, 1, :, :].rearrange("p a b -> p (a b)"),
                    in_=xt8_d[:, 2 * T : 4 * T],
                )
                nc.sync.dma_start(
                    out=w8_sb[:, 1].rearrange("p a b c -> p (a b c)"),
                    in_=w8_d[:, 1536 : 2 * 1536],
                )
                nc.scalar.dma_start(
                    out=xt8_sb[:, 2, :, :].rearrange("p a b -> p (a b)"),
                    in_=xt8_d[:, 4 * T : 6 * T],
                )
                for ftp in range(2, 6):
                    nc.sync.dma_start(
                        out=w8_sb[:, ftp].rearrange("p a b c -> p (a b c)"),
                        in_=w8_d[:, ftp * 1536 : (ftp + 1) * 1536],
                    )
                nc.gpsimd.dma_start(
                    out=wv_sb.rearrange("p a b -> p (a b)"), in_=wv_d[:, :]
                )
                for kc in range(KC):
                    eng = nc.sync if kc % 2 == 0 else nc.gpsimd
                    eng.dma_start(
                        out=xtb_sb[:, kc, :],
                        in_=xtb_d[:, kc * T : (kc + 1) * T],
                    )
                for mc in range(2):
                    nc.sync.dma_start(
                        out=at_sb[:, mc, :], in_=at_d[mc, :, :]
                    )

                with tc.tile_pool(
                    name="psP", bufs=8, space=MemorySpace.PSUM
                ) as psP:
                    emit_qkT(psP)
                    emit_v(0, psP)
                    emit_v(1, psP)

            aotp = stk.enter_context(tc.tile_pool(name="aotp", bufs=1))
            aot_new = aotp.tile([128, KC, B * TPAD], BF16)
            big['aot'] = aot_new
            with (
                tc.tile_pool(name="psu", bufs=4, space=MemorySpace.PSUM) as psu,
                tc.tile_pool(name="psSt", bufs=2, space=MemorySpace.PSUM) as psSt,
            ):
                for b in range(B):
                    open_batch(b)
                    # st groups with av blocks interleaved (exp paces PSUM
                    # recycling); uo runs one batch behind so it never waits
                    # on this batch's exp queue
                    order = [
                        ("av", 0, 0), ("st", 0, 0), ("av", 0, 1), ("st", 0, 1),
                        ("av", 1, 0), ("st", 0, 2), ("av", 1, 1), ("st", 1, 0),
                        ("vg", 0, 0), ("st", 1, 1), ("vg", 1, 0), ("st", 1, 2),
                        ("vg", 2, 0), ("vg", 3, 0),
                    ]
                    for kind, a1, a2 in order:
                        if kind == "st":
                            st_unit(b, a1, a2, psSt)
                        elif kind == "av":
                            av_unit(b, a1, a2, psu)
                        elif b + 2 < B:
                            v_group(b + 2, a1, psu)
                    if b >= 1:
                        emit_uo(b - 1, psu, ncs=(0,))
                        emit_tr(b - 1, 0)
                        emit_uo(b - 1, psu, ncs=(1,))
                        emit_tr(b - 1, 1)
                    if b >= 2:
                        emit_proj(b - 2, psu)
                    if b == 0:
                        nc.sync.dma_start(
                            out=wp_sb.rearrange("p a b -> p (a b)"),
                            in_=wp_d[:, :],
                        )
                emit_uo(B - 1, psu, ncs=(0,))
                emit_tr(B - 1, 0)
                emit_uo(B - 1, psu, ncs=(1,))
                emit_tr(B - 1, 1)
                emit_proj(B - 2, psu)
                for o in range(KC):
                    proj_unit(B - 1, o, psu, coff=0, clen=128)
                    proj_unit(B - 1, o, psu, coff=128, clen=68)

    return nc


_CACHE: dict = {}


def _get_module():
    if "nc" not in _CACHE:
        nc = bacc.Bacc(None, target_bir_lowering=False)
        _emit(nc)
        nc.compile()
        _CACHE["nc"] = nc
    return _CACHE["nc"]


def prepare_core_inputs(x_shard, W_qkv, static_a, W_proj):
    """Build the per-core input map from a [B, N, C] batch shard."""
    bf = ml_dtypes.bfloat16
    f8 = ml_dtypes.float8_e4m3
    xT = np.ascontiguousarray(
        x_shard.reshape(T, C).T
    )  # [768, 1568]
    xt8 = (
        xT.reshape(3, 2, 128, T).transpose(2, 0, 1, 3).reshape(128, 6 * T)
    ).astype(f8)
    xtb = (
        xT.reshape(KC, 128, T).transpose(1, 0, 2).reshape(128, KC * T)
    ).astype(bf)
    return dict(xt8=np.ascontiguousarray(xt8), xtb=np.ascontiguousarray(xtb))


def prepare_shared_inputs(W_qkv, static_a, W_proj):
    bf = ml_dtypes.bfloat16
    f8 = ml_dtypes.float8_e4m3
    # Permute q/k output columns into DR-pair layout: head h, dim d lands
    # at slot c*128 + p with c = 2*(h//4) + d//32, p = 32*(h%4) + d%32,
    # so st can read [32, 2, m] DR pairs from q8/k8 without a refold.
    h_i = np.arange(H)[:, None]
    d_i = np.arange(D)[None, :]
    slot = (2 * (h_i // 4) + d_i // 32) * 128 + 32 * (h_i % 4) + d_i % 32
    perm = np.empty(C, dtype=np.int64)
    perm[slot.reshape(-1)] = np.arange(C)
    Wqk = np.concatenate(
        [W_qkv[:, perm], W_qkv[:, C + perm]], axis=1
    )
    # w8[p, ftp, kp, kt, fo] = 16*Wqk[(2kp+kt)*128+p, ftp*256+fo]
    w8 = (
        (Wqk * W8SCALE)
        .reshape(3, 2, 128, 6, 256)
        .transpose(2, 3, 0, 1, 4)
        .reshape(128, 6 * 2 * C)
    ).astype(f8)
    wv = (
        W_qkv[:, 2 * C :].reshape(KC, 128, C).transpose(1, 0, 2).reshape(128, KC * C)
    ).astype(bf)
    A = static_a[0]  # [H, N, N]
    Am = np.ascontiguousarray(A.transpose(2, 0, 1))  # [m, H, n]
    at_arr = np.zeros((2, 128, H, N), dtype=np.float32)
    at_arr[0, 0:98] = Am[0:98]
    at_arr[1, 0:98] = Am[98:196]
    at = at_arr.reshape(2, 128, H * N).astype(bf)
    wp = (
        W_proj.reshape(KC, 128, C).transpose(1, 0, 2).reshape(128, KC * C)
    ).astype(bf)
    return dict(
        w8=np.ascontiguousarray(w8),
        wv=np.ascontiguousarray(wv),
        at=np.ascontiguousarray(at),
        wp=np.ascontiguousarray(wp),
    )


_last_results = None


def kernel(x, W_qkv, static_a, W_proj, b_proj):
    global _last_results
    x = np.asarray(x, dtype=np.float32)
    W_qkv = np.asarray(W_qkv, dtype=np.float32)
    static_a = np.asarray(static_a, dtype=np.float32)
    W_proj = np.asarray(W_proj, dtype=np.float32)
    b_proj = np.asarray(b_proj, dtype=np.float32)

    shared = prepare_shared_inputs(W_qkv, static_a, W_proj)
    in_maps = []
    for i in range(N_CORES):
        m = dict(shared)
        m.update(prepare_core_inputs(x[i * B : (i + 1) * B], W_qkv, static_a, W_proj))
        in_maps.append(m)

    nc = _get_module()
    res = run_bass_kernel_spmd(nc, in_maps, core_ids=list(range(N_CORES)))
    _last_results = res
    out = np.concatenate(
        [
            np.asarray(r["out"]).astype(np.float32).reshape(C, B, N).transpose(1, 2, 0)
            for r in res.results
        ],
        axis=0,
    )
    out = np.ascontiguousarray(out)
    if b_proj.any():
        out = out + b_proj.reshape(1, 1, C)
    return out

